# revision 24
# baseline (speedup 1.0000x reference)
"""ChildSum TreeLSTM (complete binary tree, 1023 nodes) on 8 trn2 NeuronCores.

Strategy: the 1023-node complete binary tree splits into 8 independent
127-node subtrees rooted at the 8 nodes of the level with 8 nodes; each
NeuronCore runs one subtree (data-parallel, zero cross-core traffic).  The
top 7 nodes are finished on the host in fp32.  On device, activations live
in transposed layout [H=300 -> 3 partition-chunks of 128, nodes] so the
level recurrence needs no transposes and child pair-sums are stride-2
free-dim adds.  Gate pre-activations (x-projections + biases) are computed
once into a persistent PSUM tensor by upfront matmuls (bias folded in via a
ones-row of the padded input), and the per-level hidden-state matmuls
accumulate into the same PSUM region, so i/u need a single fused sigmoid.
All device tensors are zero-padded to H=384 so every matmul chunk is a full
128x128 (enables the PE fast-weight-load path and leaves no uninitialized
PSUM).  Matmul operands are bf16; cell state and elementwise math are fp32.
Logits come back raw; log-softmax and the NLL loss are host-side.
"""

import os
import sys

for _p in ("/opt/trn_rl_repo",):
    if _p not in sys.path and os.path.isdir(_p):
        sys.path.append(_p)

import numpy as np
import ml_dtypes

V = 100000
E = 300
H = 300
LBL = 5
N = 1023
NCORES = 8
LPC = 64  # leaves per core
DEV_LEVELS = 4  # tree levels computed on device (leaves + 3); rest on host
LOCAL_N = 120  # nodes per core on device (64+32+16+8)
SIZES = [64, 32, 16, 8]  # per-core device level sizes
BASES = [0, 64, 96, 112]  # local level start offsets

PAD = 384  # H padded to 3 full 128-chunks
ONES_ROW = 320  # bias/ones row position inside the zero padding (chunk 2, part 64)

_compiled = {}
LAST_RESULTS = None  # BassKernelResults of the most recent device run


def _patch_cheap_tail(tile):
    """Replace Tile's kernel-tail drain + two all-engine EVSEM barriers
    (~14us) with a single gpsimd drain that waits on the global clock, then
    resets semaphores on gpsimd.  Data completeness is guaranteed by the
    drain waits; other engines simply run off the end of their streams."""
    if getattr(tile.TileContext, "_cheap_tail", False):
        return
    from concourse.vector_clock import ScopedClock, VectorClock
    from concourse.tile_scheduler import N_PROCS, PROC_NAMES

    def _drain_and_barrier(self, tick_clock, wait_clock):
        nc = self.nc
        hs = nc.alloc_semaphore("cheap_tail_hs")
        drain_inst = nc.sync.drain()
        # Exclude SWDGE queue procs from the drain waits: their completion
        # sems propagate slowly (~8us) and the gpsimd dma_reset in the sem
        # clear below drains those queues in hardware anyway.  The output
        # DMAs are issued on gpsimd SWDGE for exactly this reason.
        vals = [
            0 if PROC_NAMES[p].startswith("DMASW") else tick_clock.global_clock[p]
            for p in range(N_PROCS)
        ]
        wait_clock.add_sem_waits(
            drain_inst.ins, ScopedClock({None: VectorClock(vals)})
        )
        drain_inst.then_inc(hs, 1)
        nc.gpsimd.wait_ge(hs, 1)
        popped = nc._tile_sem_poison_stack.pop()
        assert popped is self._sem_poison
        nc.clear_and_free_semaphores(list(self.sems.allocated().values()))
        # reset the handshake sem too so the NEFF can re-execute
        nc.gpsimd.sem_clear(range(hs.num, hs.num + 1))

    tile.TileContext._drain_and_barrier = _drain_and_barrier
    tile.TileContext._cheap_tail = True


def _build_bass():
    import concourse.bacc as bacc
    import concourse.mybir as mybir
    import concourse.tile as tile

    _patch_cheap_tail(tile)

    f32 = mybir.dt.float32
    bf16 = mybir.dt.bfloat16
    AF = mybir.ActivationFunctionType

    nc = bacc.Bacc()

    # DRAM I/O (per-core shapes; SPMD with per-core in_maps)
    # inputs are host-pre-shuffled to [128, chunk*cols] so every DMA is
    # contiguous per partition (cheap descriptor generation)
    xt_d = nc.dram_tensor("xt", [128, 3 * LOCAL_N], bf16, kind="ExternalInput")
    wa_d = {
        g: nc.dram_tensor(f"wa_{g}", [128, 3 * PAD], bf16, kind="ExternalInput")
        for g in ("i", "u", "f")
    }
    wh_d = {
        g: nc.dram_tensor(f"wh_{g}", [128, 3 * PAD], bf16, kind="ExternalInput")
        for g in ("i", "u", "f")
    }
    wo_d = nc.dram_tensor("wo", [128, 3 * LBL], bf16, kind="ExternalInput")
    # single output tensor: cols [0:5] logits (rows 0..119), [5:29] level-3 h
    # (3 chunks x 8 nodes), [29:53] level-3 c
    OUTW = LBL + 6 * SIZES[3]
    out_d = nc.dram_tensor("out", [128, OUTW], f32, kind="ExternalOutput")

    with tile.TileContext(nc) as tc:
        tc.race_detector_enabled = False
        with (
            tc.tile_pool(name="const", bufs=1) as const,
            tc.tile_pool(name="state", bufs=1) as state,
            tc.tile_pool(name="scr", bufs=2) as scr,
            tc.tile_pool(name="pprex", bufs=1, space="PSUM") as pprex,
            tc.tile_pool(name="plg", bufs=1, space="PSUM") as plg,
        ):
            # ---- constants in SBUF (one consolidated DMA per tensor) ----
            xt = const.tile([128, 3, LOCAL_N], bf16, tag="xt")
            wa = {g: const.tile([128, 3, PAD], bf16, tag=f"wa{g}", name=f"wa{g}") for g in "iuf"}
            wh = {g: const.tile([128, 3, PAD], bf16, tag=f"wh{g}", name=f"wh{g}") for g in "iuf"}
            wo = const.tile([128, 3, LBL], bf16, tag="wo")

            # warm the ACT sigmoid table while DMAs run
            warm = const.tile([1, 1], f32, tag="warm")
            nc.vector.memset(warm[:, :], 0.0)
            nc.scalar.activation(out=warm[:, :], in_=warm[:, :], func=AF.Sigmoid)

            nc.gpsimd.dma_start(
                out=xt[:, :, :], in_=xt_d.rearrange("p (c n) -> p c n", c=3)
            )
            for g in "iuf":
                nc.sync.dma_start(
                    out=wa[g][:, :, :],
                    in_=wa_d[g].rearrange("p (c m) -> p c m", c=3),
                )
            # recurrence weights aren't needed until level 1; issue them on
            # the vector engine's queue so they don't serialize behind the
            # above (not gpsimd: the kernel-tail sem reset runs on gpsimd and
            # cannot prove completion of gpsimd-issued DMAs)
            for g in "iuf":
                nc.scalar.dma_start(
                    out=wh[g][:, :, :],
                    in_=wh_d[g].rearrange("p (c m) -> p c m", c=3),
                )
            nc.scalar.dma_start(
                out=wo[:, :, :], in_=wo_d.rearrange("p (c m) -> p c m", c=3)
            )

            # ---- persistent state ----
            ostage = state.tile([128, LBL + 6 * SIZES[3]], f32, tag="ostage")
            nc.vector.memset(ostage[:, :], 0.0)
            hb = state.tile([128, 3, LOCAL_N], bf16, tag="hb")  # hidden (bf16)
            cc = state.tile([128, 3, LOCAL_N], f32, tag="cc")  # cell (fp32)
            prexf = state.tile([128, 3, LOCAL_N], f32, tag="prexf")  # fxx+b copy

            # ---- PSUM ----
            # gate pre-activations, lanes (gate*3 + mchunk); gates i,u,f
            prex = pprex.tile([128, 9, 128], f32, tag="prex")
            # per-child forget pre-activation scratch
            gp = pprex.tile([128, 3, 2 * SIZES[1]], f32, tag="gp")
            lg = plg.tile([128, LBL], f32, tag="lg")

            # upfront x-projections (+bias via ones row) for all 127 nodes
            for gi, g in enumerate("iuf"):
                for mc in range(3):
                    for kc in range(3):
                        nc.tensor.matmul(
                            prex[:, gi * 3 + mc, :LOCAL_N],
                            wa[g][:, kc, mc * 128 : (mc + 1) * 128],
                            xt[:, kc, :],
                            start=(kc == 0),
                            stop=(kc == 2),
                        )

            # SBUF copy of the f-gate x-projection (fxx + b_fx + b_fh): the
            # per-level f/o adds need it as the non-PSUM operand.
            nc.vector.tensor_copy(prexf[:, :, :], prex[:, 6:9, :LOCAL_N])

            # ---- leaves: no children; i,u,o are pure x-projections ----
            iuo_l = scr.tile([128, 9, LPC], f32, tag="iuo")
            nc.scalar.activation(out=iuo_l, in_=prex[:, :, :LPC], func=AF.Sigmoid)
            nc.vector.tensor_mul(cc[:, :, :LPC], iuo_l[:, 0:3, :], iuo_l[:, 3:6, :])
            th_l = scr.tile([128, 3, LPC], f32, tag="th")
            nc.scalar.activation(out=th_l, in_=cc[:, :, :LPC], func=AF.Tanh)
            nc.vector.tensor_mul(hb[:, :, :LPC], iuo_l[:, 6:9, :], th_l)

            # ---- internal levels ----
            for lvl in range(1, DEV_LEVELS):
                P = SIZES[lvl]
                C = 2 * P
                cb = BASES[lvl - 1]
                pc = BASES[lvl]

                # child-sum of h (bf16, matmul rhs)
                hsum = scr.tile([128, 3, P], bf16, tag="hsum")
                chv = hb[:, :, cb : cb + C].rearrange(
                    "p c (n two) -> p c n two", two=2
                )
                nc.vector.tensor_add(hsum, chv[:, :, :, 0], chv[:, :, :, 1])

                # per-child forget pre-activation g = h_child @ W_fh first:
                # its elementwise consumers then overlap the i/u matmuls
                for mc in range(3):
                    for kc in range(3):
                        nc.tensor.matmul(
                            gp[:, mc, :C],
                            wh["f"][:, kc, mc * 128 : (mc + 1) * 128],
                            hb[:, kc, cb : cb + C],
                            start=(kc == 0),
                            stop=(kc == 2),
                        )

                # i, u recurrence matmuls accumulate onto the PSUM
                # pre-activations
                for gi, g in enumerate("iu"):
                    for mc in range(3):
                        for kc in range(3):
                            nc.tensor.matmul(
                                prex[:, gi * 3 + mc, pc : pc + P],
                                wh[g][:, kc, mc * 128 : (mc + 1) * 128],
                                hsum[:, kc, :],
                                start=False,
                                stop=(kc == 2),
                                skip_group_check=True,
                            )

                # fused sigmoid over i/u lanes for this level's columns
                iu = scr.tile([128, 6, P], f32, tag="iuo")
                nc.scalar.activation(
                    out=iu, in_=prex[:, 0:6, pc : pc + P], func=AF.Sigmoid
                )

                # f (per child) and o (per parent) pre-activations packed in
                # one tile -> one sigmoid:
                #   fo[:, :, :C]  = g + fxx_parent (broadcast to the pair)
                #   fo[:, :, C:]  = tmpf_even + g_odd  (= g_e + g_o + fxx)
                fo = scr.tile([128, 3, C + P], f32, tag="fo")
                gp4 = gp[:, :, :C].rearrange("p c (n two) -> p c n two", two=2)
                fo4 = fo[:, :, :C].rearrange("p c (n two) -> p c n two", two=2)
                nc.vector.tensor_add(
                    fo4, gp4, prexf[:, :, pc : pc + P].to_broadcast((128, 3, P, 2))
                )
                nc.vector.tensor_add(
                    fo[:, :, C : C + P], fo[:, :, 0:C:2], gp[:, :, 1:C:2]
                )
                fos = scr.tile([128, 3, C + P], f32, tag="fos")
                nc.scalar.activation(out=fos, in_=fo, func=AF.Sigmoid)

                # c = i*u + sum_children f*c_child ; the f-side pair sum runs
                # while the i/u sigmoid is still in flight
                fc = scr.tile([128, 3, C], f32, tag="fc")
                nc.gpsimd.tensor_mul(fc, fos[:, :, :C], cc[:, :, cb : cb + C])
                fc4 = fc.rearrange("p c (n two) -> p c n two", two=2)
                c2 = scr.tile([128, 3, P], f32, tag="c2")
                nc.gpsimd.tensor_add(c2, fc4[:, :, :, 0], fc4[:, :, :, 1])
                iup = scr.tile([128, 3, P], f32, tag="iup")
                nc.vector.tensor_mul(iup, iu[:, 0:3, :], iu[:, 3:6, :])
                nc.vector.tensor_add(cc[:, :, pc : pc + P], iup, c2)

                # h = o * tanh(c)   (stored bf16 for the next level's matmuls)
                th = scr.tile([128, 3, P], f32, tag="th")
                nc.scalar.activation(out=th, in_=cc[:, :, pc : pc + P], func=AF.Tanh)
                nc.vector.tensor_mul(hb[:, :, pc : pc + P], fos[:, :, C : C + P], th)

            # ---- logits for all 127 local nodes (softmax is host-side) ----
            # ones row at padded position 320 (= chunk 2, partition 64) so the
            # W_out matmul adds b_out.
            nc.vector.memset(hb[64:65, 2, :], 1.0)
            for kc in range(3):
                nc.tensor.matmul(
                    lg[:LOCAL_N, :],
                    hb[:, kc, :],
                    wo[:, kc, :],
                    start=(kc == 0),
                    stop=(kc == 2),
                )
            nb = BASES[3]
            nn_ = SIZES[3]
            nc.vector.tensor_copy(ostage[:LOCAL_N, 0:LBL], lg[:LOCAL_N, :])
            nc.vector.tensor_copy(
                ostage[:, LBL : LBL + 3 * nn_].rearrange("p (c n) -> p c n", c=3),
                hb[:, :, nb : nb + nn_],
            )
            nc.vector.tensor_copy(
                ostage[:, LBL + 3 * nn_ :].rearrange("p (c n) -> p c n", c=3),
                cc[:, :, nb : nb + nn_],
            )
            nc.gpsimd.dma_start(out=out_d[:, :], in_=ostage[:, :])

    nc.compile()
    return nc


def _get_compiled():
    if "nc" not in _compiled:
        _compiled["nc"] = _build_bass()
    return _compiled["nc"]


def _core_nodes(k):
    """Global node indices of core k's device-subtree, in local level order."""
    idx = []
    start, size = 0, 512
    for lvl in range(DEV_LEVELS):
        per = size // NCORES
        idx.append(np.arange(start + per * k, start + per * (k + 1)))
        start += size
        size //= 2
    return np.concatenate(idx)


def _sigmoid(z):
    return 1.0 / (1.0 + np.exp(-z))


def _make_in_maps(x, W, b):
    # shared (replicated) weight uploads, zero-padded to 384 with the
    # combined bias folded in at the ones-row position
    bf = ml_dtypes.bfloat16
    wa_np = {}
    for g, wx, whk in (("i", "ix", "ih"), ("u", "ux", "uh"), ("f", "fx", "fh")):
        m = np.zeros((PAD, PAD), np.float32)
        m[:H, :H] = W[wx]
        m[ONES_ROW, :H] = b[wx] + b[whk]
        wa_np[g] = np.ascontiguousarray(
            m.reshape(3, 128, PAD).transpose(1, 0, 2).reshape(128, 3 * PAD)
        ).astype(bf)
    wh_np = {}
    for g in "iuf":
        m = np.zeros((PAD, PAD), np.float32)
        m[:H, :H] = W[g + "h"]
        wh_np[g] = np.ascontiguousarray(
            m.reshape(3, 128, PAD).transpose(1, 0, 2).reshape(128, 3 * PAD)
        ).astype(bf)
    m = np.zeros((PAD, LBL), np.float32)
    m[:H] = W["out"]
    m[ONES_ROW] = b["out"]
    wo_np = np.ascontiguousarray(
        m.reshape(3, 128, LBL).transpose(1, 0, 2).reshape(128, 3 * LBL)
    ).astype(bf)

    node_lists = [_core_nodes(k) for k in range(NCORES)]
    in_maps = []
    for k in range(NCORES):
        xa = np.zeros((PAD, LOCAL_N), np.float32)
        xa[:300] = x[node_lists[k]].T
        xa[ONES_ROW] = 1.0
        xa = np.ascontiguousarray(
            xa.reshape(3, 128, LOCAL_N).transpose(1, 0, 2).reshape(128, 3 * LOCAL_N)
        )
        m = {"xt": xa.astype(bf), "wo": wo_np}
        for g in "iuf":
            m[f"wa_{g}"] = wa_np[g]
            m[f"wh_{g}"] = wh_np[g]
        in_maps.append(m)
    return node_lists, in_maps


def kernel(
    word_ids,
    labels,
    children_idx,
    children_mask,
    emb,
    W_ix,
    b_ix,
    W_ih,
    b_ih,
    W_fx,
    b_fx,
    W_fh,
    b_fh,
    W_ux,
    b_ux,
    W_uh,
    b_uh,
    W_out,
    b_out,
):
    global LAST_RESULTS
    from concourse import bass_utils

    word_ids = np.asarray(word_ids)
    labels = np.asarray(labels)
    children_idx = np.asarray(children_idx)
    children_mask = np.asarray(children_mask)
    emb = np.asarray(emb, dtype=np.float32)
    W = {
        "ix": np.asarray(W_ix, np.float32),
        "ih": np.asarray(W_ih, np.float32),
        "fx": np.asarray(W_fx, np.float32),
        "fh": np.asarray(W_fh, np.float32),
        "ux": np.asarray(W_ux, np.float32),
        "uh": np.asarray(W_uh, np.float32),
        "out": np.asarray(W_out, np.float32),
    }
    b = {
        "ix": np.asarray(b_ix, np.float32),
        "ih": np.asarray(b_ih, np.float32),
        "fx": np.asarray(b_fx, np.float32),
        "fh": np.asarray(b_fh, np.float32),
        "ux": np.asarray(b_ux, np.float32),
        "uh": np.asarray(b_uh, np.float32),
        "out": np.asarray(b_out, np.float32),
    }

    x = emb[word_ids]  # [1023, 300] host embedding gather
    node_lists, in_maps = _make_in_maps(x, W, b)

    nc = _get_compiled()
    res = bass_utils.run_bass_kernel_spmd(nc, in_maps, core_ids=list(range(NCORES)))
    LAST_RESULTS = res

    logits = np.empty((N, LBL), np.float32)
    # h/c for nodes from global level 3 upward (indices >= 896)
    h_all = np.zeros((N, H), np.float32)
    c_all = np.zeros((N, H), np.float32)
    for k in range(NCORES):
        out = res.results[k]["out"]
        logits[node_lists[k]] = out[:LOCAL_N, :LBL]
        # l3 slabs: [128, 3, 8] with value[p, c, j] = state[c*128+p] of node j
        l3h = out[:, LBL : LBL + 3 * SIZES[3]].reshape(128, 3, SIZES[3])
        l3c = out[:, LBL + 3 * SIZES[3] :].reshape(128, 3, SIZES[3])
        g0 = 896 + SIZES[3] * k
        for j in range(SIZES[3]):
            h_all[g0 + j] = l3h[:, :, j].T.reshape(-1)[:300]
            c_all[g0 + j] = l3c[:, :, j].T.reshape(-1)[:300]

    # ---- upper levels (global nodes 960..1022) on host, fp32, batched ----
    start = 960
    size = 32
    while size >= 1:
        nodes = np.arange(start, start + size)
        ch = children_idx[nodes]
        hl, hr = h_all[ch[:, 0]], h_all[ch[:, 1]]
        cl, cr = c_all[ch[:, 0]], c_all[ch[:, 1]]
        hsum = hl + hr
        xb = x[nodes]
        ixx = xb @ W["ix"] + b["ix"]
        fxx = xb @ W["fx"] + b["fx"]
        uxx = xb @ W["ux"] + b["ux"]
        i = _sigmoid(ixx + hsum @ W["ih"] + b["ih"])
        o = _sigmoid(fxx + hsum @ W["fh"] + b["fh"])
        u = _sigmoid(uxx + hsum @ W["uh"] + b["uh"])
        fl = _sigmoid(hl @ W["fh"] + b["fh"] + fxx)
        fr = _sigmoid(hr @ W["fh"] + b["fh"] + fxx)
        c = i * u + fl * cl + fr * cr
        h = o * np.tanh(c)
        h_all[nodes] = h
        c_all[nodes] = c
        logits[nodes] = h @ W["out"] + b["out"]
        start += size
        size //= 2

    # log-softmax + NLL on host (exact, vectorized)
    mx = logits.max(axis=1, keepdims=True)
    ex = np.exp(logits - mx)
    logp = (logits - mx) - np.log(ex.sum(axis=1, keepdims=True))
    logp = logp.astype(np.float32)
    loss = np.float32(-(logp[np.arange(N), labels].astype(np.float64).sum()))
    return logp, loss


# revision 25
# speedup vs baseline: 1.0450x; 1.0450x over previous
"""ChildSum TreeLSTM (complete binary tree, 1023 nodes) on 8 trn2 NeuronCores.

Strategy: the 1023-node complete binary tree splits into 8 independent
127-node subtrees rooted at the 8 nodes of the level with 8 nodes; each
NeuronCore runs one subtree (data-parallel, zero cross-core traffic).  The
top 7 nodes are finished on the host in fp32.  On device, activations live
in transposed layout [H=300 -> 3 partition-chunks of 128, nodes] so the
level recurrence needs no transposes and child pair-sums are stride-2
free-dim adds.  Gate pre-activations (x-projections + biases) are computed
once into a persistent PSUM tensor by upfront matmuls (bias folded in via a
ones-row of the padded input), and the per-level hidden-state matmuls
accumulate into the same PSUM region, so i/u need a single fused sigmoid.
All device tensors are zero-padded to H=384 so every matmul chunk is a full
128x128 (enables the PE fast-weight-load path and leaves no uninitialized
PSUM).  Matmul operands are bf16; cell state and elementwise math are fp32.
Logits come back raw; log-softmax and the NLL loss are host-side.
"""

import os
import sys

for _p in ("/opt/trn_rl_repo",):
    if _p not in sys.path and os.path.isdir(_p):
        sys.path.append(_p)

import numpy as np
import ml_dtypes

V = 100000
E = 300
H = 300
LBL = 5
N = 1023
NCORES = 8
LPC = 64  # leaves per core
DEV_LEVELS = 4  # tree levels computed on device (leaves + 3); rest on host
LOCAL_N = 120  # nodes per core on device (64+32+16+8)
SIZES = [64, 32, 16, 8]  # per-core device level sizes
BASES = [0, 64, 96, 112]  # local level start offsets

PAD = 384  # H padded to 3 full 128-chunks
ONES_ROW = 320  # bias/ones row position inside the zero padding (chunk 2, part 64)

_compiled = {}
LAST_RESULTS = None  # BassKernelResults of the most recent device run


def _patch_cheap_tail(tile):
    """Replace Tile's kernel-tail drain + two all-engine EVSEM barriers
    (~14us) with a single gpsimd drain that waits on the global clock, then
    resets semaphores on gpsimd.  Data completeness is guaranteed by the
    drain waits; other engines simply run off the end of their streams."""
    if getattr(tile.TileContext, "_cheap_tail", False):
        return
    from concourse.vector_clock import ScopedClock, VectorClock
    from concourse.tile_scheduler import N_PROCS, PROC_NAMES

    def _drain_and_barrier(self, tick_clock, wait_clock):
        nc = self.nc
        hs = nc.alloc_semaphore("cheap_tail_hs")
        drain_inst = nc.sync.drain()
        # Exclude SWDGE queue procs from the drain waits: their completion
        # sems propagate slowly (~8us) and the gpsimd dma_reset in the sem
        # clear below drains those queues in hardware anyway.  The output
        # DMAs are issued on gpsimd SWDGE for exactly this reason.
        vals = [
            0 if PROC_NAMES[p].startswith("DMASW") else tick_clock.global_clock[p]
            for p in range(N_PROCS)
        ]
        wait_clock.add_sem_waits(
            drain_inst.ins, ScopedClock({None: VectorClock(vals)})
        )
        drain_inst.then_inc(hs, 1)
        nc.gpsimd.wait_ge(hs, 1)
        popped = nc._tile_sem_poison_stack.pop()
        assert popped is self._sem_poison
        nc.clear_and_free_semaphores(list(self.sems.allocated().values()))
        # reset the handshake sem too so the NEFF can re-execute
        nc.gpsimd.sem_clear(range(hs.num, hs.num + 1))

    tile.TileContext._drain_and_barrier = _drain_and_barrier
    tile.TileContext._cheap_tail = True


def _build_bass():
    import concourse.bacc as bacc
    import concourse.mybir as mybir
    import concourse.tile as tile

    _patch_cheap_tail(tile)

    f32 = mybir.dt.float32
    bf16 = mybir.dt.bfloat16
    AF = mybir.ActivationFunctionType

    nc = bacc.Bacc()

    # DRAM I/O (per-core shapes; SPMD with per-core in_maps)
    # inputs are host-pre-shuffled to [128, chunk*cols] so every DMA is
    # contiguous per partition (cheap descriptor generation)
    xt_d = nc.dram_tensor("xt", [128, 3 * LOCAL_N], bf16, kind="ExternalInput")
    wa_d = {
        g: nc.dram_tensor(f"wa_{g}", [128, 3 * PAD], bf16, kind="ExternalInput")
        for g in ("i", "u", "f")
    }
    wh_d = {
        g: nc.dram_tensor(f"wh_{g}", [128, 3 * PAD], bf16, kind="ExternalInput")
        for g in ("i", "u", "f")
    }
    wo_d = nc.dram_tensor("wo", [128, 3 * LBL], bf16, kind="ExternalInput")
    # single output tensor: cols [0:5] logits (rows 0..119), [5:29] level-3 h
    # (3 chunks x 8 nodes), [29:53] level-3 c
    OUTW = LBL + 6 * SIZES[3]
    out_d = nc.dram_tensor("out", [128, OUTW], f32, kind="ExternalOutput")

    with tile.TileContext(nc) as tc:
        tc.race_detector_enabled = False
        with (
            tc.tile_pool(name="const", bufs=1) as const,
            tc.tile_pool(name="state", bufs=1) as state,
            tc.tile_pool(name="scr", bufs=2) as scr,
            tc.tile_pool(name="pprex", bufs=1, space="PSUM") as pprex,
            tc.tile_pool(name="plg", bufs=1, space="PSUM") as plg,
        ):
            # ---- constants in SBUF (one consolidated DMA per tensor) ----
            xt = const.tile([128, 3, LOCAL_N], bf16, tag="xt")
            wa = {g: const.tile([128, 3, PAD], bf16, tag=f"wa{g}", name=f"wa{g}") for g in "iuf"}
            wh = {g: const.tile([128, 3, PAD], bf16, tag=f"wh{g}", name=f"wh{g}") for g in "iuf"}
            wo = const.tile([128, 3, LBL], bf16, tag="wo")

            # warm the ACT sigmoid table while DMAs run
            warm = const.tile([1, 1], f32, tag="warm")
            nc.vector.memset(warm[:, :], 0.0)
            nc.scalar.activation(out=warm[:, :], in_=warm[:, :], func=AF.Sigmoid)

            nc.gpsimd.dma_start(
                out=xt[:, :, :], in_=xt_d.rearrange("p (c n) -> p c n", c=3)
            )
            for g in "iuf":
                nc.sync.dma_start(
                    out=wa[g][:, :, :],
                    in_=wa_d[g].rearrange("p (c m) -> p c m", c=3),
                )
            # recurrence weights aren't needed until level 1; issue them on
            # the vector engine's queue so they don't serialize behind the
            # above (not gpsimd: the kernel-tail sem reset runs on gpsimd and
            # cannot prove completion of gpsimd-issued DMAs)
            for g in "iuf":
                nc.scalar.dma_start(
                    out=wh[g][:, :, :],
                    in_=wh_d[g].rearrange("p (c m) -> p c m", c=3),
                )
            nc.scalar.dma_start(
                out=wo[:, :, :], in_=wo_d.rearrange("p (c m) -> p c m", c=3)
            )

            # ---- persistent state ----
            ostage = state.tile([128, LBL + 6 * SIZES[3]], f32, tag="ostage")
            nc.vector.memset(ostage[:, :], 0.0)
            hb = state.tile([128, 3, LOCAL_N], bf16, tag="hb")  # hidden (bf16)
            cc = state.tile([128, 3, LOCAL_N], f32, tag="cc")  # cell (fp32)
            prexf = state.tile([128, 3, LOCAL_N], f32, tag="prexf")  # fxx+b copy

            # ---- PSUM ----
            # gate pre-activations, lanes (gate*3 + mchunk); gates i,u,f
            prex = pprex.tile([128, 9, 128], f32, tag="prex")
            # per-child forget pre-activation scratch
            gp = pprex.tile([128, 3, 2 * SIZES[1]], f32, tag="gp")
            lg = plg.tile([128, LBL], f32, tag="lg")

            # upfront x-projections (+bias via ones row) for all 127 nodes
            for gi, g in enumerate("iuf"):
                for mc in range(3):
                    for kc in range(3):
                        nc.tensor.matmul(
                            prex[:, gi * 3 + mc, :LOCAL_N],
                            wa[g][:, kc, mc * 128 : (mc + 1) * 128],
                            xt[:, kc, :],
                            start=(kc == 0),
                            stop=(kc == 2),
                        )

            # SBUF copy of the f-gate x-projection (fxx + b_fx + b_fh): the
            # per-level f/o adds need it as the non-PSUM operand.
            nc.vector.tensor_copy(prexf[:, :, :], prex[:, 6:9, :LOCAL_N])

            # ---- leaves: no children; i,u,o are pure x-projections ----
            iuo_l = scr.tile([128, 9, LPC], f32, tag="iuo")
            nc.scalar.activation(
                out=iuo_l[:, 0:6, :], in_=prex[:, 0:6, :LPC], func=AF.Sigmoid
            )
            nc.vector.tensor_mul(cc[:, :, :LPC], iuo_l[:, 0:3, :], iuo_l[:, 3:6, :])
            nc.scalar.activation(
                out=iuo_l[:, 6:9, :], in_=prex[:, 6:9, :LPC], func=AF.Sigmoid
            )
            th_l = scr.tile([128, 3, LPC], f32, tag="th")
            nc.scalar.activation(out=th_l, in_=cc[:, :, :LPC], func=AF.Tanh)
            nc.vector.tensor_mul(hb[:, :, :LPC], iuo_l[:, 6:9, :], th_l)

            # ---- internal levels ----
            for lvl in range(1, DEV_LEVELS):
                P = SIZES[lvl]
                C = 2 * P
                cb = BASES[lvl - 1]
                pc = BASES[lvl]

                # child-sum of h (bf16, matmul rhs)
                hsum = scr.tile([128, 3, P], bf16, tag="hsum")
                chv = hb[:, :, cb : cb + C].rearrange(
                    "p c (n two) -> p c n two", two=2
                )
                nc.vector.tensor_add(hsum, chv[:, :, :, 0], chv[:, :, :, 1])

                # per-child forget pre-activation g = h_child @ W_fh first:
                # its elementwise consumers then overlap the i/u matmuls
                for mc in range(3):
                    for kc in range(3):
                        nc.tensor.matmul(
                            gp[:, mc, :C],
                            wh["f"][:, kc, mc * 128 : (mc + 1) * 128],
                            hb[:, kc, cb : cb + C],
                            start=(kc == 0),
                            stop=(kc == 2),
                        )

                # i, u recurrence matmuls accumulate onto the PSUM
                # pre-activations
                for gi, g in enumerate("iu"):
                    for mc in range(3):
                        for kc in range(3):
                            nc.tensor.matmul(
                                prex[:, gi * 3 + mc, pc : pc + P],
                                wh[g][:, kc, mc * 128 : (mc + 1) * 128],
                                hsum[:, kc, :],
                                start=False,
                                stop=(kc == 2),
                                skip_group_check=True,
                            )

                # fused sigmoid over i/u lanes for this level's columns
                iu = scr.tile([128, 6, P], f32, tag="iuo")
                nc.scalar.activation(
                    out=iu, in_=prex[:, 0:6, pc : pc + P], func=AF.Sigmoid
                )

                # f (per child) and o (per parent) pre-activations packed in
                # one tile -> one sigmoid:
                #   fo[:, :, :C]  = g + fxx_parent (broadcast to the pair)
                #   fo[:, :, C:]  = tmpf_even + g_odd  (= g_e + g_o + fxx)
                fo = scr.tile([128, 3, C + P], f32, tag="fo")
                gp4 = gp[:, :, :C].rearrange("p c (n two) -> p c n two", two=2)
                fo4 = fo[:, :, :C].rearrange("p c (n two) -> p c n two", two=2)
                nc.vector.tensor_add(
                    fo4, gp4, prexf[:, :, pc : pc + P].to_broadcast((128, 3, P, 2))
                )
                nc.vector.tensor_add(
                    fo[:, :, C : C + P], fo[:, :, 0:C:2], gp[:, :, 1:C:2]
                )
                fos = scr.tile([128, 3, C + P], f32, tag="fos")
                nc.scalar.activation(out=fos, in_=fo, func=AF.Sigmoid)

                # c = i*u + sum_children f*c_child ; the f-side pair sum runs
                # while the i/u sigmoid is still in flight
                fc = scr.tile([128, 3, C], f32, tag="fc")
                nc.vector.tensor_mul(fc, fos[:, :, :C], cc[:, :, cb : cb + C])
                fc4 = fc.rearrange("p c (n two) -> p c n two", two=2)
                c2 = scr.tile([128, 3, P], f32, tag="c2")
                nc.vector.tensor_add(c2, fc4[:, :, :, 0], fc4[:, :, :, 1])
                iup = scr.tile([128, 3, P], f32, tag="iup")
                nc.vector.tensor_mul(iup, iu[:, 0:3, :], iu[:, 3:6, :])
                nc.vector.tensor_add(cc[:, :, pc : pc + P], iup, c2)

                # h = o * tanh(c)   (stored bf16 for the next level's matmuls)
                th = scr.tile([128, 3, P], f32, tag="th")
                nc.scalar.activation(out=th, in_=cc[:, :, pc : pc + P], func=AF.Tanh)
                nc.vector.tensor_mul(hb[:, :, pc : pc + P], fos[:, :, C : C + P], th)

            # ---- logits for all 127 local nodes (softmax is host-side) ----
            # ones row at padded position 320 (= chunk 2, partition 64) so the
            # W_out matmul adds b_out.
            nc.vector.memset(hb[64:65, 2, :], 1.0)
            for kc in range(3):
                nc.tensor.matmul(
                    lg[:LOCAL_N, :],
                    hb[:, kc, :],
                    wo[:, kc, :],
                    start=(kc == 0),
                    stop=(kc == 2),
                )
            nb = BASES[3]
            nn_ = SIZES[3]
            nc.vector.tensor_copy(ostage[:LOCAL_N, 0:LBL], lg[:LOCAL_N, :])
            nc.vector.tensor_copy(
                ostage[:, LBL : LBL + 3 * nn_].rearrange("p (c n) -> p c n", c=3),
                hb[:, :, nb : nb + nn_],
            )
            nc.vector.tensor_copy(
                ostage[:, LBL + 3 * nn_ :].rearrange("p (c n) -> p c n", c=3),
                cc[:, :, nb : nb + nn_],
            )
            nc.gpsimd.dma_start(out=out_d[:, :], in_=ostage[:, :])

    nc.compile()
    return nc


def _get_compiled():
    if "nc" not in _compiled:
        _compiled["nc"] = _build_bass()
    return _compiled["nc"]


def _core_nodes(k):
    """Global node indices of core k's device-subtree, in local level order."""
    idx = []
    start, size = 0, 512
    for lvl in range(DEV_LEVELS):
        per = size // NCORES
        idx.append(np.arange(start + per * k, start + per * (k + 1)))
        start += size
        size //= 2
    return np.concatenate(idx)


def _sigmoid(z):
    return 1.0 / (1.0 + np.exp(-z))


def _make_in_maps(x, W, b):
    # shared (replicated) weight uploads, zero-padded to 384 with the
    # combined bias folded in at the ones-row position
    bf = ml_dtypes.bfloat16
    wa_np = {}
    for g, wx, whk in (("i", "ix", "ih"), ("u", "ux", "uh"), ("f", "fx", "fh")):
        m = np.zeros((PAD, PAD), np.float32)
        m[:H, :H] = W[wx]
        m[ONES_ROW, :H] = b[wx] + b[whk]
        wa_np[g] = np.ascontiguousarray(
            m.reshape(3, 128, PAD).transpose(1, 0, 2).reshape(128, 3 * PAD)
        ).astype(bf)
    wh_np = {}
    for g in "iuf":
        m = np.zeros((PAD, PAD), np.float32)
        m[:H, :H] = W[g + "h"]
        wh_np[g] = np.ascontiguousarray(
            m.reshape(3, 128, PAD).transpose(1, 0, 2).reshape(128, 3 * PAD)
        ).astype(bf)
    m = np.zeros((PAD, LBL), np.float32)
    m[:H] = W["out"]
    m[ONES_ROW] = b["out"]
    wo_np = np.ascontiguousarray(
        m.reshape(3, 128, LBL).transpose(1, 0, 2).reshape(128, 3 * LBL)
    ).astype(bf)

    node_lists = [_core_nodes(k) for k in range(NCORES)]
    in_maps = []
    for k in range(NCORES):
        xa = np.zeros((PAD, LOCAL_N), np.float32)
        xa[:300] = x[node_lists[k]].T
        xa[ONES_ROW] = 1.0
        xa = np.ascontiguousarray(
            xa.reshape(3, 128, LOCAL_N).transpose(1, 0, 2).reshape(128, 3 * LOCAL_N)
        )
        m = {"xt": xa.astype(bf), "wo": wo_np}
        for g in "iuf":
            m[f"wa_{g}"] = wa_np[g]
            m[f"wh_{g}"] = wh_np[g]
        in_maps.append(m)
    return node_lists, in_maps


def kernel(
    word_ids,
    labels,
    children_idx,
    children_mask,
    emb,
    W_ix,
    b_ix,
    W_ih,
    b_ih,
    W_fx,
    b_fx,
    W_fh,
    b_fh,
    W_ux,
    b_ux,
    W_uh,
    b_uh,
    W_out,
    b_out,
):
    global LAST_RESULTS
    from concourse import bass_utils

    word_ids = np.asarray(word_ids)
    labels = np.asarray(labels)
    children_idx = np.asarray(children_idx)
    children_mask = np.asarray(children_mask)
    emb = np.asarray(emb, dtype=np.float32)
    W = {
        "ix": np.asarray(W_ix, np.float32),
        "ih": np.asarray(W_ih, np.float32),
        "fx": np.asarray(W_fx, np.float32),
        "fh": np.asarray(W_fh, np.float32),
        "ux": np.asarray(W_ux, np.float32),
        "uh": np.asarray(W_uh, np.float32),
        "out": np.asarray(W_out, np.float32),
    }
    b = {
        "ix": np.asarray(b_ix, np.float32),
        "ih": np.asarray(b_ih, np.float32),
        "fx": np.asarray(b_fx, np.float32),
        "fh": np.asarray(b_fh, np.float32),
        "ux": np.asarray(b_ux, np.float32),
        "uh": np.asarray(b_uh, np.float32),
        "out": np.asarray(b_out, np.float32),
    }

    x = emb[word_ids]  # [1023, 300] host embedding gather
    node_lists, in_maps = _make_in_maps(x, W, b)

    nc = _get_compiled()
    res = bass_utils.run_bass_kernel_spmd(nc, in_maps, core_ids=list(range(NCORES)))
    LAST_RESULTS = res

    logits = np.empty((N, LBL), np.float32)
    # h/c for nodes from global level 3 upward (indices >= 896)
    h_all = np.zeros((N, H), np.float32)
    c_all = np.zeros((N, H), np.float32)
    for k in range(NCORES):
        out = res.results[k]["out"]
        logits[node_lists[k]] = out[:LOCAL_N, :LBL]
        # l3 slabs: [128, 3, 8] with value[p, c, j] = state[c*128+p] of node j
        l3h = out[:, LBL : LBL + 3 * SIZES[3]].reshape(128, 3, SIZES[3])
        l3c = out[:, LBL + 3 * SIZES[3] :].reshape(128, 3, SIZES[3])
        g0 = 896 + SIZES[3] * k
        for j in range(SIZES[3]):
            h_all[g0 + j] = l3h[:, :, j].T.reshape(-1)[:300]
            c_all[g0 + j] = l3c[:, :, j].T.reshape(-1)[:300]

    # ---- upper levels (global nodes 960..1022) on host, fp32, batched ----
    start = 960
    size = 32
    while size >= 1:
        nodes = np.arange(start, start + size)
        ch = children_idx[nodes]
        hl, hr = h_all[ch[:, 0]], h_all[ch[:, 1]]
        cl, cr = c_all[ch[:, 0]], c_all[ch[:, 1]]
        hsum = hl + hr
        xb = x[nodes]
        ixx = xb @ W["ix"] + b["ix"]
        fxx = xb @ W["fx"] + b["fx"]
        uxx = xb @ W["ux"] + b["ux"]
        i = _sigmoid(ixx + hsum @ W["ih"] + b["ih"])
        o = _sigmoid(fxx + hsum @ W["fh"] + b["fh"])
        u = _sigmoid(uxx + hsum @ W["uh"] + b["uh"])
        fl = _sigmoid(hl @ W["fh"] + b["fh"] + fxx)
        fr = _sigmoid(hr @ W["fh"] + b["fh"] + fxx)
        c = i * u + fl * cl + fr * cr
        h = o * np.tanh(c)
        h_all[nodes] = h
        c_all[nodes] = c
        logits[nodes] = h @ W["out"] + b["out"]
        start += size
        size //= 2

    # log-softmax + NLL on host (exact, vectorized)
    mx = logits.max(axis=1, keepdims=True)
    ex = np.exp(logits - mx)
    logp = (logits - mx) - np.log(ex.sum(axis=1, keepdims=True))
    logp = logp.astype(np.float32)
    loss = np.float32(-(logp[np.arange(N), labels].astype(np.float64).sum()))
    return logp, loss


# revision 26
# speedup vs baseline: 1.0895x; 1.0426x over previous
"""ChildSum TreeLSTM (complete binary tree, 1023 nodes) on 8 trn2 NeuronCores.

Strategy: the 1023-node complete binary tree splits into 8 independent
subtrees; each NeuronCore runs one subtree's bottom 4 levels (960 of 1023
nodes, data-parallel, zero cross-core traffic).  The remaining 63 top nodes
(levels with <= 4 nodes per core) are finished on the host in fp32, where
the per-level batches are too small to use the hardware.  On device,
activations live
in transposed layout [H=300 -> 3 partition-chunks of 128, nodes] so the
level recurrence needs no transposes and child pair-sums are stride-2
free-dim adds.  Gate pre-activations (x-projections + biases) are computed
once into a persistent PSUM tensor by upfront matmuls (bias folded in via a
ones-row of the padded input), and the per-level hidden-state matmuls
accumulate into the same PSUM region, so i/u need a single fused sigmoid.
All device tensors are zero-padded to H=384 so every matmul chunk is a full
128x128 (enables the PE fast-weight-load path and leaves no uninitialized
PSUM).  Matmul operands are bf16; cell state and elementwise math are fp32.
Logits come back raw; log-softmax and the NLL loss are host-side.
"""

import os
import sys

for _p in ("/opt/trn_rl_repo",):
    if _p not in sys.path and os.path.isdir(_p):
        sys.path.append(_p)

import numpy as np
import ml_dtypes

V = 100000
E = 300
H = 300
LBL = 5
N = 1023
NCORES = 8
LPC = 64  # leaves per core
DEV_LEVELS = 4  # tree levels computed on device (leaves + 3); rest on host
LOCAL_N = 120  # nodes per core on device (64+32+16+8)
SIZES = [64, 32, 16, 8]  # per-core device level sizes
BASES = [0, 64, 96, 112]  # local level start offsets

PAD = 384  # H padded to 3 full 128-chunks
ONES_ROW = 320  # bias/ones row position inside the zero padding (chunk 2, part 64)

_compiled = {}
LAST_RESULTS = None  # BassKernelResults of the most recent device run


def _patch_cheap_tail(tile):
    """Replace Tile's kernel-tail drain + two all-engine EVSEM barriers
    (~14us) with a single gpsimd drain that waits on the global clock, then
    resets semaphores on gpsimd.  Data completeness is guaranteed by the
    drain waits; other engines simply run off the end of their streams."""
    if getattr(tile.TileContext, "_cheap_tail", False):
        return
    from concourse.vector_clock import ScopedClock, VectorClock
    from concourse.tile_scheduler import N_PROCS, PROC_NAMES

    def _drain_and_barrier(self, tick_clock, wait_clock):
        nc = self.nc
        hs = nc.alloc_semaphore("cheap_tail_hs")
        drain_inst = nc.sync.drain()
        # Exclude SWDGE queue procs from the drain waits: their completion
        # sems propagate slowly (~8us) and the gpsimd dma_reset in the sem
        # clear below drains those queues in hardware anyway.  The output
        # DMAs are issued on gpsimd SWDGE for exactly this reason.
        vals = [
            0 if PROC_NAMES[p].startswith("DMASW") else tick_clock.global_clock[p]
            for p in range(N_PROCS)
        ]
        wait_clock.add_sem_waits(
            drain_inst.ins, ScopedClock({None: VectorClock(vals)})
        )
        drain_inst.then_inc(hs, 1)
        nc.gpsimd.wait_ge(hs, 1)
        popped = nc._tile_sem_poison_stack.pop()
        assert popped is self._sem_poison
        nc.clear_and_free_semaphores(list(self.sems.allocated().values()))
        # reset the handshake sem too so the NEFF can re-execute
        nc.gpsimd.sem_clear(range(hs.num, hs.num + 1))

    tile.TileContext._drain_and_barrier = _drain_and_barrier
    tile.TileContext._cheap_tail = True


def _build_bass():
    import concourse.bacc as bacc
    import concourse.mybir as mybir
    import concourse.tile as tile

    _patch_cheap_tail(tile)

    f32 = mybir.dt.float32
    bf16 = mybir.dt.bfloat16
    AF = mybir.ActivationFunctionType

    nc = bacc.Bacc()

    # DRAM I/O (per-core shapes; SPMD with per-core in_maps)
    # inputs are host-pre-shuffled to [128, chunk*cols] so every DMA is
    # contiguous per partition (cheap descriptor generation)
    xt_d = nc.dram_tensor("xt", [128, 3 * LOCAL_N], bf16, kind="ExternalInput")
    wa_d = {
        g: nc.dram_tensor(f"wa_{g}", [128, 3 * PAD], bf16, kind="ExternalInput")
        for g in ("i", "u", "f")
    }
    wh_d = {
        g: nc.dram_tensor(f"wh_{g}", [128, 3 * PAD], bf16, kind="ExternalInput")
        for g in ("i", "u", "f")
    }
    wo_d = nc.dram_tensor("wo", [128, 3 * LBL], bf16, kind="ExternalInput")
    # single output tensor: cols [0:5] logits (rows 0..119), [5:29] level-3 h
    # (3 chunks x 8 nodes), [29:53] level-3 c
    OUTW = LBL + 6 * SIZES[3]
    out_d = nc.dram_tensor("out", [128, OUTW], f32, kind="ExternalOutput")

    with tile.TileContext(nc) as tc:
        tc.race_detector_enabled = False
        with (
            tc.tile_pool(name="const", bufs=1) as const,
            tc.tile_pool(name="state", bufs=1) as state,
            tc.tile_pool(name="scr", bufs=2) as scr,
            tc.tile_pool(name="pprex", bufs=1, space="PSUM") as pprex,
            tc.tile_pool(name="plg", bufs=1, space="PSUM") as plg,
        ):
            # ---- constants in SBUF (one consolidated DMA per tensor) ----
            xt = const.tile([128, 3, LOCAL_N], bf16, tag="xt")
            wa = {g: const.tile([128, 3, PAD], bf16, tag=f"wa{g}", name=f"wa{g}") for g in "iuf"}
            wh = {g: const.tile([128, 3, PAD], bf16, tag=f"wh{g}", name=f"wh{g}") for g in "iuf"}
            wo = const.tile([128, 3, LBL], bf16, tag="wo")

            # warm the ACT sigmoid table while DMAs run
            warm = const.tile([1, 1], f32, tag="warm")
            nc.vector.memset(warm[:, :], 0.0)
            nc.scalar.activation(out=warm[:, :], in_=warm[:, :], func=AF.Sigmoid)

            nc.gpsimd.dma_start(
                out=xt[:, :, :], in_=xt_d.rearrange("p (c n) -> p c n", c=3)
            )
            for g in "iuf":
                nc.sync.dma_start(
                    out=wa[g][:, :, :],
                    in_=wa_d[g].rearrange("p (c m) -> p c m", c=3),
                )
            # recurrence weights aren't needed until level 1; issue them on
            # the vector engine's queue so they don't serialize behind the
            # above (not gpsimd: the kernel-tail sem reset runs on gpsimd and
            # cannot prove completion of gpsimd-issued DMAs)
            for g in "iuf":
                nc.scalar.dma_start(
                    out=wh[g][:, :, :],
                    in_=wh_d[g].rearrange("p (c m) -> p c m", c=3),
                )
            nc.scalar.dma_start(
                out=wo[:, :, :], in_=wo_d.rearrange("p (c m) -> p c m", c=3)
            )

            # ---- persistent state ----
            ostage = state.tile([128, LBL + 6 * SIZES[3]], f32, tag="ostage")
            nc.vector.memset(ostage[:, :], 0.0)
            hb = state.tile([128, 3, LOCAL_N], bf16, tag="hb")  # hidden (bf16)
            cc = state.tile([128, 3, LOCAL_N], f32, tag="cc")  # cell (fp32)
            prexf = state.tile([128, 3, LOCAL_N], f32, tag="prexf")  # fxx+b copy

            # ---- PSUM ----
            # gate pre-activations, lanes (gate*3 + mchunk); gates i,u,f
            prex = pprex.tile([128, 9, 128], f32, tag="prex")
            # per-child forget pre-activation scratch
            gp = pprex.tile([128, 3, 2 * SIZES[1]], f32, tag="gp")
            lg = plg.tile([128, LBL], f32, tag="lg")

            # upfront x-projections (+bias via ones row) for all 127 nodes
            for gi, g in enumerate("iuf"):
                for mc in range(3):
                    for kc in range(3):
                        nc.tensor.matmul(
                            prex[:, gi * 3 + mc, :LOCAL_N],
                            wa[g][:, kc, mc * 128 : (mc + 1) * 128],
                            xt[:, kc, :],
                            start=(kc == 0),
                            stop=(kc == 2),
                        )

            # SBUF copy of the f-gate x-projection (fxx + b_fx + b_fh): the
            # per-level f/o adds need it as the non-PSUM operand.
            nc.vector.tensor_copy(prexf[:, :, :], prex[:, 6:9, :LOCAL_N])

            # ---- leaves: no children; i,u,o are pure x-projections ----
            iuo_l = scr.tile([128, 9, LPC], f32, tag="iuo")
            nc.scalar.activation(
                out=iuo_l[:, 0:6, :], in_=prex[:, 0:6, :LPC], func=AF.Sigmoid
            )
            nc.vector.tensor_mul(cc[:, :, :LPC], iuo_l[:, 0:3, :], iuo_l[:, 3:6, :])
            nc.scalar.activation(
                out=iuo_l[:, 6:9, :], in_=prex[:, 6:9, :LPC], func=AF.Sigmoid
            )
            th_l = scr.tile([128, 3, LPC], f32, tag="th")
            nc.scalar.activation(out=th_l, in_=cc[:, :, :LPC], func=AF.Tanh)
            nc.vector.tensor_mul(hb[:, :, :LPC], iuo_l[:, 6:9, :], th_l)

            # ---- internal levels ----
            for lvl in range(1, DEV_LEVELS):
                P = SIZES[lvl]
                C = 2 * P
                cb = BASES[lvl - 1]
                pc = BASES[lvl]

                # child-sum of h (bf16, matmul rhs)
                hsum = scr.tile([128, 3, P], bf16, tag="hsum")
                chv = hb[:, :, cb : cb + C].rearrange(
                    "p c (n two) -> p c n two", two=2
                )
                nc.vector.tensor_add(hsum, chv[:, :, :, 0], chv[:, :, :, 1])

                # per-child forget pre-activation g = h_child @ W_fh first:
                # its elementwise consumers then overlap the i/u matmuls
                for mc in range(3):
                    for kc in range(3):
                        nc.tensor.matmul(
                            gp[:, mc, :C],
                            wh["f"][:, kc, mc * 128 : (mc + 1) * 128],
                            hb[:, kc, cb : cb + C],
                            start=(kc == 0),
                            stop=(kc == 2),
                        )

                # i, u recurrence matmuls accumulate onto the PSUM
                # pre-activations
                for gi, g in enumerate("iu"):
                    for mc in range(3):
                        for kc in range(3):
                            nc.tensor.matmul(
                                prex[:, gi * 3 + mc, pc : pc + P],
                                wh[g][:, kc, mc * 128 : (mc + 1) * 128],
                                hsum[:, kc, :],
                                start=False,
                                stop=(kc == 2),
                                skip_group_check=True,
                            )

                # fused sigmoid over i/u lanes for this level's columns
                iu = scr.tile([128, 6, P], f32, tag="iuo")
                nc.scalar.activation(
                    out=iu, in_=prex[:, 0:6, pc : pc + P], func=AF.Sigmoid
                )

                # f (per child) and o (per parent) pre-activations packed in
                # one tile -> one sigmoid:
                #   fo[:, :, :C]  = g + fxx_parent (broadcast to the pair)
                #   fo[:, :, C:]  = tmpf_even + g_odd  (= g_e + g_o + fxx)
                fo = scr.tile([128, 3, C + P], f32, tag="fo")
                gp4 = gp[:, :, :C].rearrange("p c (n two) -> p c n two", two=2)
                fo4 = fo[:, :, :C].rearrange("p c (n two) -> p c n two", two=2)
                nc.vector.tensor_add(
                    fo4, gp4, prexf[:, :, pc : pc + P].to_broadcast((128, 3, P, 2))
                )
                nc.vector.tensor_add(
                    fo[:, :, C : C + P], fo[:, :, 0:C:2], gp[:, :, 1:C:2]
                )
                fos = scr.tile([128, 3, C + P], f32, tag="fos")
                nc.scalar.activation(out=fos, in_=fo, func=AF.Sigmoid)

                # c = i*u + sum_children f*c_child ; the f-side pair sum runs
                # while the i/u sigmoid is still in flight
                fc = scr.tile([128, 3, C], f32, tag="fc")
                nc.vector.tensor_mul(fc, fos[:, :, :C], cc[:, :, cb : cb + C])
                fc4 = fc.rearrange("p c (n two) -> p c n two", two=2)
                c2 = scr.tile([128, 3, P], f32, tag="c2")
                nc.vector.tensor_add(c2, fc4[:, :, :, 0], fc4[:, :, :, 1])
                iup = scr.tile([128, 3, P], f32, tag="iup")
                nc.vector.tensor_mul(iup, iu[:, 0:3, :], iu[:, 3:6, :])
                nc.vector.tensor_add(cc[:, :, pc : pc + P], iup, c2)

                # h = o * tanh(c)   (stored bf16 for the next level's matmuls)
                th = scr.tile([128, 3, P], f32, tag="th")
                nc.scalar.activation(out=th, in_=cc[:, :, pc : pc + P], func=AF.Tanh)
                nc.vector.tensor_mul(hb[:, :, pc : pc + P], fos[:, :, C : C + P], th)

            # ---- logits for all 127 local nodes (softmax is host-side) ----
            # ones row at padded position 320 (= chunk 2, partition 64) so the
            # W_out matmul adds b_out.
            nc.vector.memset(hb[64:65, 2, :], 1.0)
            for kc in range(3):
                nc.tensor.matmul(
                    lg[:LOCAL_N, :],
                    hb[:, kc, :],
                    wo[:, kc, :],
                    start=(kc == 0),
                    stop=(kc == 2),
                )
            nb = BASES[3]
            nn_ = SIZES[3]
            nc.vector.tensor_copy(ostage[:LOCAL_N, 0:LBL], lg[:LOCAL_N, :])
            nc.vector.tensor_copy(
                ostage[:, LBL : LBL + 3 * nn_].rearrange("p (c n) -> p c n", c=3),
                hb[:, :, nb : nb + nn_],
            )
            nc.vector.tensor_copy(
                ostage[:, LBL + 3 * nn_ :].rearrange("p (c n) -> p c n", c=3),
                cc[:, :, nb : nb + nn_],
            )
            nc.gpsimd.dma_start(out=out_d[:, :], in_=ostage[:, :])

    nc.compile()
    return nc


def _get_compiled():
    if "nc" not in _compiled:
        _compiled["nc"] = _build_bass()
    return _compiled["nc"]


def _core_nodes(k):
    """Global node indices of core k's device-subtree, in local level order."""
    idx = []
    start, size = 0, 512
    for lvl in range(DEV_LEVELS):
        per = size // NCORES
        idx.append(np.arange(start + per * k, start + per * (k + 1)))
        start += size
        size //= 2
    return np.concatenate(idx)


def _sigmoid(z):
    return 1.0 / (1.0 + np.exp(-z))


def _make_in_maps(x, W, b):
    # shared (replicated) weight uploads, zero-padded to 384 with the
    # combined bias folded in at the ones-row position
    bf = ml_dtypes.bfloat16
    wa_np = {}
    for g, wx, whk in (("i", "ix", "ih"), ("u", "ux", "uh"), ("f", "fx", "fh")):
        m = np.zeros((PAD, PAD), np.float32)
        m[:H, :H] = W[wx]
        m[ONES_ROW, :H] = b[wx] + b[whk]
        wa_np[g] = np.ascontiguousarray(
            m.reshape(3, 128, PAD).transpose(1, 0, 2).reshape(128, 3 * PAD)
        ).astype(bf)
    wh_np = {}
    for g in "iuf":
        m = np.zeros((PAD, PAD), np.float32)
        m[:H, :H] = W[g + "h"]
        wh_np[g] = np.ascontiguousarray(
            m.reshape(3, 128, PAD).transpose(1, 0, 2).reshape(128, 3 * PAD)
        ).astype(bf)
    m = np.zeros((PAD, LBL), np.float32)
    m[:H] = W["out"]
    m[ONES_ROW] = b["out"]
    wo_np = np.ascontiguousarray(
        m.reshape(3, 128, LBL).transpose(1, 0, 2).reshape(128, 3 * LBL)
    ).astype(bf)

    node_lists = [_core_nodes(k) for k in range(NCORES)]
    in_maps = []
    for k in range(NCORES):
        xa = np.zeros((PAD, LOCAL_N), np.float32)
        xa[:300] = x[node_lists[k]].T
        xa[ONES_ROW] = 1.0
        xa = np.ascontiguousarray(
            xa.reshape(3, 128, LOCAL_N).transpose(1, 0, 2).reshape(128, 3 * LOCAL_N)
        )
        m = {"xt": xa.astype(bf), "wo": wo_np}
        for g in "iuf":
            m[f"wa_{g}"] = wa_np[g]
            m[f"wh_{g}"] = wh_np[g]
        in_maps.append(m)
    return node_lists, in_maps


def kernel(
    word_ids,
    labels,
    children_idx,
    children_mask,
    emb,
    W_ix,
    b_ix,
    W_ih,
    b_ih,
    W_fx,
    b_fx,
    W_fh,
    b_fh,
    W_ux,
    b_ux,
    W_uh,
    b_uh,
    W_out,
    b_out,
):
    global LAST_RESULTS
    from concourse import bass_utils

    word_ids = np.asarray(word_ids)
    labels = np.asarray(labels)
    children_idx = np.asarray(children_idx)
    children_mask = np.asarray(children_mask)
    emb = np.asarray(emb, dtype=np.float32)
    W = {
        "ix": np.asarray(W_ix, np.float32),
        "ih": np.asarray(W_ih, np.float32),
        "fx": np.asarray(W_fx, np.float32),
        "fh": np.asarray(W_fh, np.float32),
        "ux": np.asarray(W_ux, np.float32),
        "uh": np.asarray(W_uh, np.float32),
        "out": np.asarray(W_out, np.float32),
    }
    b = {
        "ix": np.asarray(b_ix, np.float32),
        "ih": np.asarray(b_ih, np.float32),
        "fx": np.asarray(b_fx, np.float32),
        "fh": np.asarray(b_fh, np.float32),
        "ux": np.asarray(b_ux, np.float32),
        "uh": np.asarray(b_uh, np.float32),
        "out": np.asarray(b_out, np.float32),
    }

    x = emb[word_ids]  # [1023, 300] host embedding gather
    node_lists, in_maps = _make_in_maps(x, W, b)

    nc = _get_compiled()
    res = bass_utils.run_bass_kernel_spmd(nc, in_maps, core_ids=list(range(NCORES)))
    LAST_RESULTS = res

    logits = np.empty((N, LBL), np.float32)
    # h/c for nodes from global level 3 upward (indices >= 896)
    h_all = np.zeros((N, H), np.float32)
    c_all = np.zeros((N, H), np.float32)
    for k in range(NCORES):
        out = res.results[k]["out"]
        logits[node_lists[k]] = out[:LOCAL_N, :LBL]
        # l3 slabs: [128, 3, 8] with value[p, c, j] = state[c*128+p] of node j
        l3h = out[:, LBL : LBL + 3 * SIZES[3]].reshape(128, 3, SIZES[3])
        l3c = out[:, LBL + 3 * SIZES[3] :].reshape(128, 3, SIZES[3])
        g0 = 896 + SIZES[3] * k
        for j in range(SIZES[3]):
            h_all[g0 + j] = l3h[:, :, j].T.reshape(-1)[:300]
            c_all[g0 + j] = l3c[:, :, j].T.reshape(-1)[:300]

    # ---- upper levels (global nodes 960..1022) on host, fp32, batched ----
    start = 960
    size = 32
    while size >= 1:
        nodes = np.arange(start, start + size)
        ch = children_idx[nodes]
        hl, hr = h_all[ch[:, 0]], h_all[ch[:, 1]]
        cl, cr = c_all[ch[:, 0]], c_all[ch[:, 1]]
        hsum = hl + hr
        xb = x[nodes]
        ixx = xb @ W["ix"] + b["ix"]
        fxx = xb @ W["fx"] + b["fx"]
        uxx = xb @ W["ux"] + b["ux"]
        i = _sigmoid(ixx + hsum @ W["ih"] + b["ih"])
        o = _sigmoid(fxx + hsum @ W["fh"] + b["fh"])
        u = _sigmoid(uxx + hsum @ W["uh"] + b["uh"])
        fl = _sigmoid(hl @ W["fh"] + b["fh"] + fxx)
        fr = _sigmoid(hr @ W["fh"] + b["fh"] + fxx)
        c = i * u + fl * cl + fr * cr
        h = o * np.tanh(c)
        h_all[nodes] = h
        c_all[nodes] = c
        logits[nodes] = h @ W["out"] + b["out"]
        start += size
        size //= 2

    # log-softmax + NLL on host (exact, vectorized)
    mx = logits.max(axis=1, keepdims=True)
    ex = np.exp(logits - mx)
    logp = (logits - mx) - np.log(ex.sum(axis=1, keepdims=True))
    logp = logp.astype(np.float32)
    loss = np.float32(-(logp[np.arange(N), labels].astype(np.float64).sum()))
    return logp, loss
